# revision 18
# baseline (speedup 1.0000x reference)
"""Trainium2 Bass kernel for CantorGlobalAttention (sparse routed attention).

Strategy: routes are shared across batch and heads, so the sparse
gather-attention is reformulated as dense matmuls using a host-precomputed
route-multiplicity matrix m[s,j] = #{k: routes[s,k] = j}:

    out[s] = (sum_j m[s,j] exp(SC[s,j]) v[j]) / (sum_j m[s,j] exp(SC[s,j]))
    SC = q @ k^T / sqrt(HD)

Transposed layout (feature dim on partitions):
    qkT[n,s]  = (W_qk^T x^T)              (W stationary)
    SCT[j,s]  = k^T(j-tile)^T q^T         (K=64 matmul, head-pair row-packed)
    ET        = mT * exp(SCT)             (ACT exp; or a 16-bit Schraudolph
                                           exp on DVE: u16(psc + lnm-bias
                                           table) reinterpreted as bf16)
    o2T       = [v|1]-block ^T @ ET       (Z replicated on one partition half)
    outT      = o2T * (1/Z)               (recip + mult; rz crosses halves
                                           via SBUF DMA)
    y[s,n]    = outT^T @ W_proj(rows)     (per-core partial, bf16 out)

Schedule: one flat software pipeline over 128 steps (pair, sc, jt):
SC(jt) -> exp/em(jt) -> AV(jt-2), with the AV lag crossing sc boundaries so
the PE never drains (HAM stays warm).  qkv-projection waves, v-extraction,
and output-projection pieces are slotted into fixed step windows as PE
filler.  DMA arrival is sequenced as an s-major wavefront so the first SC
fires ~10us in.  exp work is split ACT / DVE(Schraudolph) / GpSimd(mult) to
keep every engine under the PE's streaming floor.

Sharding: 8 cores = 2 batches x 4 head-groups (4 heads each). Host sums the
4 per-batch partials and adds b_proj (and the dropped v-bias term exactly).
"""

import numpy as np
import ml_dtypes
from contextlib import ExitStack

import concourse.bacc as bacc
import concourse.mybir as mybir
import concourse.tile as tile
from concourse.bass import ts
from concourse.bass_utils import run_bass_kernel_spmd

bf16 = ml_dtypes.bfloat16
F32 = mybir.dt.float32
BF16 = mybir.dt.bfloat16
U16 = mybir.dt.uint16
I16 = mybir.dt.int16
Alu = mybir.AluOpType
Act = mybir.ActivationFunctionType

B, S, D = 2, 2048, 1024
H, HD, K = 16, 64, 64
NCORES = 8
HG = 4            # head-groups (cores per batch)
NH = H // HG      # heads per core = 4
DH = NH * HD      # feature cols per core for q/k/v = 256
ST = S // 128     # 16 s-tiles
JT = S // 128     # 16 j-tiles
KT = D // 128     # 8 contraction tiles for the projections

# --- exp configuration -------------------------------------------------
A16 = 128.0 / np.log(2.0)          # bf16-bits per unit natural-log
AQ = A16 / 8.0                     # q prescale folding 1/sqrt(HD)
B16 = 16251.0                      # Schraudolph bias (127<<7 - tuning)
LNM_NEG = -20000.0                 # m=0 sentinel (wrap- and saturate-safe)
SCHRAUD = True                     # jt%4==3 tiles use DVE Schraudolph exp
GPSIMD_MULT = True                 # jt%4==1 tiles (after sc0) mult on GpSimd

_CACHED_NC = None
_LAST_RESULTS = None


def _build_bass():
    nc = bacc.Bacc("TRN2", target_bir_lowering=False, debug=False)

    xT_d = nc.dram_tensor("xT", [128, KT, S], BF16, kind="ExternalInput")
    wqk_d = nc.dram_tensor("wqk", [6, 128, KT * 128], BF16, kind="ExternalInput")
    wproj_d = nc.dram_tensor("wproj", [128, 2, D], BF16, kind="ExternalInput")
    mt_d = nc.dram_tensor("mt", [128, JT, S], BF16, kind="ExternalInput")
    bqk_d = nc.dram_tensor("bqk", [128, 6], F32, kind="ExternalInput")
    y_d = nc.dram_tensor("y", [KT, 128, S], BF16, kind="ExternalOutput")

    exp_scale = (1.0 / A16) if SCHRAUD else 0.125

    def is_schraud(jt):
        return SCHRAUD and jt % 4 == 3 and jt != 15

    def em_engine(scg, jt):
        if GPSIMD_MULT and scg > 0 and jt % 4 == 1:
            return nc.gpsimd
        return nc.vector

    with tile.TileContext(nc) as tc, ExitStack() as ctx:
        cp = ctx.enter_context(tc.tile_pool(name="consts", bufs=1))

        wqk_sb = cp.tile([128, KT, 3 * DH], BF16)
        wproj_sb = cp.tile([128, 2, D], BF16)
        mt_sb = cp.tile([128, JT, S], BF16)
        bqk_sb = cp.tile([128, 6], F32)
        ones_f32 = cp.tile([128, 128], F32)
        qkT_sb = cp.tile([128, 4, S], BF16)  # nt 0,1 = qT; 2,3 = kT
        # [p, st, pair, hip, 128]: hip0 = [v|ones], hip1 = [ones|v]
        vext_sb = cp.tile([128, ST, 2, 2, 128], BF16)
        outT_sb = cp.tile([128, 2, S], BF16)     # proj lhsT layout

        xtp = tc.alloc_tile_pool(name="xtp", bufs=1)
        xT_sb = xtp.tile([128, KT, S], BF16)

        # ---- DMA wavefront: s-major so the pipeline starts early -------
        # scalar (HWDGE) queue: small consts + sc0 m-table
        nc.scalar.dma_start(bqk_sb[:], bqk_d[:])

        def load_wqk_block(eng, nt):
            eng.dma_start(
                wqk_sb[:, :, ts(nt, 128)],
                wqk_d[nt].rearrange("p (k c) -> p k c", c=128))

        load_wqk_block(nc.scalar, 2)      # k pair-0 weights first
        load_wqk_block(nc.scalar, 0)      # q pair-0

        # sync queue: xT in s-chunks, all kt of sq0 first
        for sq in range(4):
            for kt in range(KT):
                nc.sync.dma_start(xT_sb[:, kt, ts(sq, 512)],
                                  xT_d[:, kt, ts(sq, 512)])

        # scalar: m-table chunks for sc0, v weights
        for j4 in range(0, JT, 4):
            nc.scalar.dma_start(mt_sb[:, j4:j4 + 4, 0:512],
                                mt_d[:, j4:j4 + 4, 0:512])
        load_wqk_block(nc.scalar, 4)
        load_wqk_block(nc.scalar, 5)

        # sync queue, after all xT issues: later m-table chunks + pair-1
        # weights + wproj -- the issue serialization keeps them from
        # stealing HBM bandwidth from the xT wavefront
        for sc in range(1, 4):
            for j4 in range(0, JT, 4):
                nc.sync.dma_start(mt_sb[:, j4:j4 + 4, ts(sc, 512)],
                                  mt_d[:, j4:j4 + 4, ts(sc, 512)])
            if sc == 1:
                load_wqk_block(nc.sync, 3)
                load_wqk_block(nc.sync, 1)
            if sc == 2:
                nc.sync.dma_start(wproj_sb[:], wproj_d[:])

        # ---- constants + engine warmups during the DMA window ----------
        nc.vector.memset(ones_f32[:], 1.0)
        ones_bf = cp.tile([128, 128], BF16)
        nc.vector.memset(ones_bf[:], 1.0)
        nc.vector.memset(vext_sb[:, :, :, 0, HD:128], 1.0)
        nc.vector.memset(vext_sb[:, :, :, 1, 0:HD], 1.0)
        tblwarm = cp.tile([1, 8], F32)
        nc.scalar.activation(tblwarm[:], ones_f32[0:1, 0:8], Act.Exp)
        if GPSIMD_MULT:
            gwarm = cp.tile([1, 8], BF16)
            nc.gpsimd.tensor_tensor(gwarm[:], ones_bf[0:1, 0:8],
                                    ones_bf[0:1, 0:8], Alu.mult)

        pwarm = tc.alloc_tile_pool(name="pwarm", bufs=1, space="PSUM")
        warm = pwarm.tile([128, 128], F32, tag="warm", bufs=1)
        for _ in range(40):
            nc.tensor.matmul(warm[:], ones_bf[:], ones_bf[:],
                             start=True, stop=True, skip_group_check=True)
        pwarm.release()

        # ---- prologue: kT/qT for sq0 (kt-waves chasing the xT DMA) -----
        pbp = tc.alloc_tile_pool(name="pbp", bufs=1, space="PSUM")
        pk0 = pbp.tile([128, 512], F32, tag="pk0")
        pq0 = pbp.tile([128, 512], F32, tag="pq0")
        for kt in range(KT):
            nc.tensor.matmul(pk0[:], wqk_sb[:, kt, ts(2, 128)],
                             xT_sb[:, kt, 0:512],
                             start=(kt == 0), stop=(kt == KT - 1))
            nc.tensor.matmul(pq0[:], wqk_sb[:, kt, ts(0, 128)],
                             xT_sb[:, kt, 0:512],
                             start=(kt == 0), stop=(kt == KT - 1))
        nc.vector.tensor_scalar(qkT_sb[:, 2, 0:512], pk0[:],
                                bqk_sb[:, 2:3], None, Alu.add)
        if SCHRAUD:
            nc.vector.tensor_scalar(qkT_sb[:, 0, 0:512], pq0[:],
                                    bqk_sb[:, 0:1], AQ, Alu.add, Alu.mult)
        else:
            nc.vector.tensor_scalar(qkT_sb[:, 0, 0:512], pq0[:],
                                    bqk_sb[:, 0:1], None, Alu.add)
        pbp.release()

        dwork = tc.alloc_tile_pool(name="dwork", bufs=3)
        ystage = tc.alloc_tile_pool(name="ystage", bufs=3)
        pscp = tc.alloc_tile_pool(name="pscp", bufs=2, space="PSUM")
        po2p = tc.alloc_tile_pool(name="po2p", bufs=1, space="PSUM")
        aux = tc.alloc_tile_pool(name="aux0", bufs=1, space="PSUM")

        # ---- v extraction (c units): 8 kt-MMs + 2 casts ----------------
        def emit_c(st):
            pv = aux.tile([128, DH], F32, tag="pv", name=f"pv{st}")
            for kt in range(KT):
                nc.tensor.matmul(
                    pv[:], xT_sb[:, kt, ts(st, 128)],
                    wqk_sb[:, kt, 4 * 128:6 * 128],
                    start=(kt == 0), stop=(kt == KT - 1))
            pvv = pv[:].rearrange("p (pr hp d) -> p pr hp d", pr=2, hp=2)
            nc.vector.tensor_copy(vext_sb[:, st, :, 0, 0:HD], pvv[:, :, 0, :])
            nc.vector.tensor_copy(vext_sb[:, st, :, 1, HD:128], pvv[:, :, 1, :])

        emit_c(0)
        emit_c(1)

        # ---- qkv-projection waves (b units), 2 kt-MMs per step ---------
        bstate = {}

        def emit_b_piece(nt, sq, phase):
            if phase == 0:
                bstate[(nt, sq)] = aux.tile([128, 512], F32, tag="pq2",
                                            name=f"pq2_{nt}_{sq}")
            pq2 = bstate[(nt, sq)]
            for kt in range(2 * phase, 2 * phase + 2):
                nc.tensor.matmul(
                    pq2[:], wqk_sb[:, kt, ts(nt, 128)],
                    xT_sb[:, kt, ts(sq, 512)],
                    start=(kt == 0), stop=(kt == KT - 1))
            if phase == 3:
                if nt in (0, 1) and SCHRAUD:
                    nc.vector.tensor_scalar(
                        qkT_sb[:, nt, ts(sq, 512)], pq2[:],
                        bqk_sb[:, nt:nt + 1], AQ, Alu.add, Alu.mult)
                else:
                    nc.vector.tensor_scalar(
                        qkT_sb[:, nt, ts(sq, 512)], pq2[:],
                        bqk_sb[:, nt:nt + 1], None, Alu.add)

        # unit -> first global step; 4 consecutive steps each
        BSCHED = {
            (2, 1): 0, (2, 2): 4, (2, 3): 8,
            (0, 1): 12, (0, 2): 16, (0, 3): 20,
            (3, 0): 24, (3, 1): 28, (3, 2): 32, (3, 3): 36,
            (1, 0): 40, (1, 1): 44, (1, 2): 48, (1, 3): 52,
        }
        bsteps = {}
        for (nt, sq), st0 in BSCHED.items():
            for ph in range(4):
                bsteps.setdefault(st0 + ph, []).append((nt, sq, ph))

        # ---- output projection pieces ----------------------------------
        pstate = {}

        def emit_proj_mm(sc, g, phase, pool):
            nt = 2 * g + phase
            slot = pstate[(g, phase)] = pool.tile(
                [128, 512], F32, tag=f"py{phase}", name=f"py_{sc}_{nt}")
            for it in range(2):
                nc.tensor.matmul(
                    slot[:], wproj_sb[:, it, ts(nt, 128)],
                    outT_sb[:, it, ts(sc, 512)],
                    start=(it == 0), stop=(it == 1))

        def emit_proj_evac(sc, g):
            py0 = pstate.pop((g, 0))
            py1 = pstate.pop((g, 1))
            ysb = ystage.tile([128, 1024], BF16, tag="ysb", name=f"ysb{sc}_{g}")
            nc.scalar.copy(ysb[:, 0:512], py0[:])
            nc.vector.tensor_copy(ysb[:, 512:1024], py1[:])
            nc.sync.dma_start(y_d[2 * g, :, ts(sc, 512)], ysb[:, 0:512])
            nc.sync.dma_start(y_d[2 * g + 1, :, ts(sc, 512)], ysb[:, 512:1024])

        # ---- per-step primitives ---------------------------------------
        def emit_sc_mm(pair, sc, jt, warmfill=False):
            psc = pscp.tile([128, 1024], F32, tag="psc", name=f"psc{pair}{sc}{jt}")
            qT2 = qkT_sb[:, pair, :]
            kT2 = qkT_sb[:, 2 + pair, :]
            if warmfill:
                nc.tensor.matmul(psc[:, 0:512], ones_bf[:],
                                 qkT_sb[:, 0, 0:512], start=True, stop=True,
                                 skip_group_check=True)
            nc.tensor.matmul(psc[:, 0:512], kT2[0:64, ts(jt, 128)],
                             qT2[0:64, ts(sc, 512)], start=True, stop=True)
            nc.tensor.matmul(psc[:, 512:1024], kT2[64:128, ts(jt, 128)],
                             qT2[64:128, ts(sc, 512)], start=True, stop=True)
            return psc

        def emit_em(scg, sc, jt, psc):
            s0 = sc * 512
            em = dwork.tile([128, 1024], BF16, tag="em", bufs=6,
                            name=f"em{scg}_{jt}")
            if is_schraud(jt):
                lnm = mt_sb[:, jt, s0:s0 + 512].bitcast(I16)
                mrep = lnm.unsqueeze(1).broadcast_to((128, 2, 512))
                nc.vector.tensor_tensor(
                    em[:].bitcast(U16).rearrange("p (t s) -> p t s", t=2),
                    psc[:].rearrange("p (t s) -> p t s", t=2),
                    mrep, Alu.add)
            else:
                eb = dwork.tile([128, 1024], BF16, tag="eb", bufs=3,
                                name=f"eb{scg}_{jt}")
                nc.scalar.activation(eb[:], psc[:], Act.Exp, scale=exp_scale)
                eng = em_engine(scg, jt)
                mtc = mt_sb[:, jt, s0:s0 + 512]
                eng.tensor_tensor(em[:, 0:512], eb[:, 0:512], mtc, Alu.mult)
                eng.tensor_tensor(em[:, 512:1024], eb[:, 512:1024], mtc, Alu.mult)
            return em

        po2state = {}

        def emit_av(pair, scg, jt, em):
            po2a, po2b = po2state[scg]
            nc.tensor.matmul(po2a[:], vext_sb[:, jt, pair, 0, :],
                             em[:, 0:512],
                             start=(jt == 0), stop=(jt == JT - 1))
            nc.tensor.matmul(po2b[:], vext_sb[:, jt, pair, 1, :],
                             em[:, 512:1024],
                             start=(jt == 0), stop=(jt == JT - 1))

        def emit_norm(pair, sc, scg, piece):
            """normalize po2(scg) into outT; piece 0..3 across steps."""
            po2a, po2b = po2state[scg]
            s0 = sc * 512
            it = pair
            st = po2state.setdefault(("n", scg), {})
            if piece == 0:
                st["o2sa"] = dwork.tile([128, 512], F32, tag="o2sa", bufs=2, name=f"o2sa{scg}")
                st["o2sb"] = dwork.tile([128, 512], F32, tag="o2sb", bufs=2, name=f"o2sb{scg}")
                nc.scalar.copy(st["o2sa"][:], po2a[:])
                nc.scalar.copy(st["o2sb"][:], po2b[:])
            elif piece == 1:
                st["ra"] = dwork.tile([128, 512], F32, tag="rza", bufs=2, name=f"ra{scg}")
                st["rb"] = dwork.tile([128, 512], F32, tag="rzb", bufs=2, name=f"rb{scg}")
                nc.vector.reciprocal_approx_fast(out=st["ra"][:],
                                                 in_=st["o2sa"][:])
                nc.sync.dma_start(st["ra"][0:64, :], st["ra"][64:128, :])
                nc.vector.reciprocal_approx_fast(out=st["rb"][:],
                                                 in_=st["o2sb"][:])
                nc.sync.dma_start(st["rb"][64:128, :], st["rb"][0:64, :])
            elif piece == 2:
                nc.vector.tensor_tensor(
                    outT_sb[0:64, it, s0:s0 + 512],
                    st["o2sa"][0:64, :], st["ra"][0:64, :], Alu.mult)
                nc.vector.tensor_tensor(
                    outT_sb[64:128, it, s0:s0 + 512],
                    st["o2sb"][64:128, :], st["rb"][64:128, :], Alu.mult)
                del po2state[("n", scg)]

        # ---- the flat pipeline -----------------------------------------
        steps = [(pair, sc, jt)
                 for pair in range(2) for sc in range(4) for jt in range(JT)]
        pend = {}      # global step -> (pair, scg, jt, em)
        aux_released = False
        projp = None

        for g, (pair, sc, jt) in enumerate(steps):
            scg = pair * 4 + sc
            lstep = jt                     # local step within this sc

            # release phase-A aux slots once the last b unit is done;
            # the proj pool takes over the banks
            if g == 57 and not aux_released:
                aux.release()
                projp = tc.alloc_tile_pool(name="auxP", bufs=1, space="PSUM")
                aux_released = True

            # normalize pieces for the previous sc
            if scg > 0 and 3 <= lstep <= 5:
                emit_norm(pair if sc > 0 else 1 - pair,
                          sc - 1 if sc > 0 else 3, scg - 1, lstep - 3)

            # output projection for sc-1 during pair-1 (local steps 6..13)
            if pair == 1 and sc > 0 and 6 <= lstep <= 13:
                pg, pph = divmod(lstep - 6, 2)
                if pph == 0:
                    emit_proj_mm(sc - 1, pg, 0, projp)
                else:
                    emit_proj_mm(sc - 1, pg, 1, projp)
                    emit_proj_evac(sc - 1, pg)

            # fresh po2 accumulators at the start of each sc
            if jt == 0:
                po2a = po2p.tile([128, 512], F32, tag="po2a", name=f"po2a{scg}")
                po2b = po2p.tile([128, 512], F32, tag="po2b", name=f"po2b{scg}")
                po2state[scg] = (po2a, po2b)

            # AV with lag 3 (crosses sc boundaries -> PE never drains;
            # 3 steps of slack keep the exp/em chain off the PE's back)
            if g - 3 in pend:
                p_pair, p_scg, p_jt, p_em = pend.pop(g - 3)
                emit_av(p_pair, p_scg, p_jt, p_em)

            psc = emit_sc_mm(pair, sc, jt, warmfill=(scg > 0 and lstep <= 1))
            em = emit_em(scg, sc, jt, psc)
            pend[g] = (pair, scg, jt, em)

            # DMA-gated filler last, so it never head-of-line-blocks the
            # PE queue ahead of the independent SC/AV stream
            if scg == 0 and 1 <= lstep <= 14:
                emit_c(lstep + 1)
            for (nt, sq, ph) in bsteps.get(g, ()):
                emit_b_piece(nt, sq, ph)

        # ---- tail: last two AVs, last normalize, last projection -------
        # stack order: projp is on top -> it donates its banks to the tail
        projp.release()
        tailp = tc.alloc_tile_pool(name="tailp", bufs=1, space="PSUM")

        for gg in (125, 126, 127):
            p_pair, p_scg, p_jt, p_em = pend.pop(gg)
            emit_av(p_pair, p_scg, p_jt, p_em)

        s3 = 3 * 512
        tail0 = {}
        for nt in range(2):
            slot = tail0[nt] = tailp.tile([128, 512], F32, tag=f"pyt{nt}",
                                          name=f"pyt0_{nt}")
            nc.tensor.matmul(slot[:], wproj_sb[:, 0, ts(nt, 128)],
                             outT_sb[:, 0, s3:s3 + 512], start=True, stop=False)

        # normalize sc3 of pair-1 (staged, same as the in-loop pattern)
        po2a, po2b = po2state[7]
        o2ta = dwork.tile([128, 512], F32, tag="o2sa", bufs=2, name="o2ta")
        o2tb = dwork.tile([128, 512], F32, tag="o2sb", bufs=2, name="o2tb")
        nc.scalar.copy(o2ta[:], po2a[:])
        nc.vector.tensor_copy(o2tb[:], po2b[:])
        ra = dwork.tile([128, 512], F32, tag="rza", bufs=2, name="ratail")
        rb = dwork.tile([128, 512], F32, tag="rzb", bufs=2, name="rbtail")
        nc.vector.reciprocal_approx_fast(out=ra[:], in_=o2ta[:])
        nc.sync.dma_start(ra[0:64, :], ra[64:128, :])
        nc.vector.reciprocal_approx_fast(out=rb[:], in_=o2tb[:])
        nc.sync.dma_start(rb[64:128, :], rb[0:64, :])
        nc.vector.tensor_tensor(outT_sb[0:64, 1, s3:s3 + 512],
                                o2ta[0:64, :], ra[0:64, :], Alu.mult)
        nc.vector.tensor_tensor(outT_sb[64:128, 1, s3:s3 + 512],
                                o2tb[64:128, :], rb[64:128, :], Alu.mult)

        # finish the projection: it=1 accumulation + evac, two slots
        def tail_finish(nt, slot):
            nc.tensor.matmul(slot[:], wproj_sb[:, 1, ts(nt, 128)],
                             outT_sb[:, 1, s3:s3 + 512], start=False, stop=True)
            ysb = ystage.tile([128, 512], BF16, tag="ysb2", bufs=4,
                              name=f"ysbt{nt}")
            if nt % 2 == 0:
                nc.scalar.copy(ysb[:], slot[:])
            else:
                nc.vector.tensor_copy(ysb[:], slot[:])
            nc.sync.dma_start(y_d[nt, :, s3:s3 + 512], ysb[:])

        for nt in range(2):
            tail_finish(nt, tail0.pop(nt))
        for nt in range(2, 8):
            slot = tailp.tile([128, 512], F32, tag=f"pyt{nt % 2}",
                              name=f"pyt_{nt}")
            nc.tensor.matmul(slot[:], wproj_sb[:, 0, ts(nt, 128)],
                             outT_sb[:, 0, s3:s3 + 512], start=True, stop=False)
            tail_finish(nt, slot)

        tailp.release()
        po2p.release()
        pscp.release()
        ystage.release()
        dwork.release()
        xtp.release()

    nc.compile()
    return nc


def _get_nc():
    global _CACHED_NC
    if _CACHED_NC is None:
        _CACHED_NC = _build_bass()
    return _CACHED_NC


def _routes_payload(routes):
    """m-multiplicity table, transposed; Schraudolph rows carry the int16
    lnm-bias payload instead of bf16 m."""
    r = np.clip(np.asarray(routes).astype(np.int64), 0, S - 1)
    m = np.zeros((S, S), dtype=np.float64)
    np.add.at(m, (np.arange(S)[:, None].repeat(K, 1).ravel(), r.ravel()), 1.0)
    mT = m.T  # [j, s]
    payload = np.ascontiguousarray(mT.astype(bf16).reshape(JT, 128, S)
                                   .transpose(1, 0, 2))
    if SCHRAUD:
        lnm = np.where(mT > 0, A16 * np.log(np.maximum(mT, 1e-9)) + B16,
                       LNM_NEG)
        lnm_i16 = np.round(lnm).astype(np.int16)
        lnm_i16 = np.ascontiguousarray(lnm_i16.reshape(JT, 128, S)
                                       .transpose(1, 0, 2))
        for jt in range(JT):
            if jt % 4 == 3:
                payload[:, jt, :] = lnm_i16[:, jt, :].view(bf16)
    return payload


def _prep_core_inputs(x, W_qkv, b_qkv, W_proj, payload):
    maps = []
    for core in range(NCORES):
        b, hg = core // HG, core % HG
        c0 = hg * DH
        xT = np.ascontiguousarray(x[b].T).astype(bf16)            # (1024, 2048)
        wqk = np.concatenate(
            [W_qkv[:, c0:c0 + DH], W_qkv[:, D + c0:D + c0 + DH],
             W_qkv[:, 2 * D + c0:2 * D + c0 + DH]], axis=1)        # (1024, 768)
        bqk = np.concatenate([b_qkv[c0:c0 + DH], b_qkv[D + c0:D + c0 + DH],
                              b_qkv[2 * D + c0:2 * D + c0 + DH]])
        wproj = W_proj[c0:c0 + DH, :]                              # (256, 1024)
        maps.append({
            "xT": np.ascontiguousarray(xT.reshape(KT, 128, S).transpose(1, 0, 2)),
            "wqk": np.ascontiguousarray(
                wqk.astype(bf16).reshape(KT, 128, 6, 128)
                .transpose(2, 1, 0, 3).reshape(6, 128, KT * 128)),
            "wproj": np.ascontiguousarray(
                wproj.astype(bf16).reshape(2, 128, D).transpose(1, 0, 2)),
            "mt": payload,
            "bqk": np.ascontiguousarray(
                bqk.astype(np.float32).reshape(6, 128).T),
        })
    return maps


def kernel(x, W_qkv, b_qkv, W_proj, b_proj, routes):
    x = np.asarray(x, dtype=np.float32)
    W_qkv = np.asarray(W_qkv, dtype=np.float32)
    b_qkv = np.asarray(b_qkv, dtype=np.float32)
    W_proj = np.asarray(W_proj, dtype=np.float32)
    b_proj = np.asarray(b_proj, dtype=np.float32)

    payload = _routes_payload(routes)
    nc = _get_nc()
    in_maps = _prep_core_inputs(x, W_qkv, b_qkv, W_proj, payload)
    res = run_bass_kernel_spmd(nc, in_maps, core_ids=list(range(NCORES)))
    global _LAST_RESULTS
    _LAST_RESULTS = res

    y = np.zeros((B, S, D), dtype=np.float32)
    for core in range(NCORES):
        b = core // HG
        yT = res.results[core]["y"]          # (KT, 128, S) bf16, transposed
        y[b] += yT.astype(np.float32).reshape(D, S).T
    # device drops the v-bias; softmax weights sum to 1 so it adds exactly
    # b_v @ W_proj to every position
    y += (b_proj + b_qkv[2 * D:3 * D] @ W_proj)[None, None, :]
    return y


# revision 19
# speedup vs baseline: 1.0969x; 1.0969x over previous
"""Trainium2 Bass kernel for CantorGlobalAttention (sparse routed attention).

Strategy: routes are shared across batch and heads, so the sparse
gather-attention is reformulated as dense matmuls using a host-precomputed
route-multiplicity matrix m[s,j] = #{k: routes[s,k] = j}:

    out[s] = (sum_j m[s,j] exp(SC[s,j]) v[j]) / (sum_j m[s,j] exp(SC[s,j]))
    SC = q @ k^T / sqrt(HD)

Transposed layout (feature dim on partitions):
    qkT[n,s]  = (W_qk^T x^T)              (W stationary)
    SCT[j,s]  = k^T(j-tile)^T q^T         (K=64 matmul, head-pair row-packed)
    ET        = mT * exp(SCT)             (ACT exp; or a 16-bit Schraudolph
                                           exp on DVE: u16(psc + lnm-bias
                                           table) reinterpreted as bf16)
    o2T       = [v|1]-block ^T @ ET       (Z replicated on one partition half)
    outT      = o2T * (1/Z)               (recip + mult; rz crosses halves
                                           via SBUF DMA)
    y[s,n]    = outT^T @ W_proj(rows)     (per-core partial, bf16 out)

Schedule: one flat software pipeline over 128 steps (pair, sc, jt):
SC(jt) -> exp/em(jt) -> AV(jt-2), with the AV lag crossing sc boundaries so
the PE never drains (HAM stays warm).  qkv-projection waves, v-extraction,
and output-projection pieces are slotted into fixed step windows as PE
filler.  DMA arrival is sequenced as an s-major wavefront so the first SC
fires ~10us in.  exp work is split ACT / DVE(Schraudolph) / GpSimd(mult) to
keep every engine under the PE's streaming floor.

Sharding: 8 cores = 2 batches x 4 head-groups (4 heads each). Host sums the
4 per-batch partials and adds b_proj (and the dropped v-bias term exactly).
"""

import numpy as np
import ml_dtypes
from contextlib import ExitStack

import concourse.bacc as bacc
import concourse.mybir as mybir
import concourse.tile as tile
from concourse.bass import ts
from concourse.bass_utils import run_bass_kernel_spmd

bf16 = ml_dtypes.bfloat16
F32 = mybir.dt.float32
BF16 = mybir.dt.bfloat16
U16 = mybir.dt.uint16
I16 = mybir.dt.int16
Alu = mybir.AluOpType
Act = mybir.ActivationFunctionType

B, S, D = 2, 2048, 1024
H, HD, K = 16, 64, 64
NCORES = 8
HG = 4            # head-groups (cores per batch)
NH = H // HG      # heads per core = 4
DH = NH * HD      # feature cols per core for q/k/v = 256
ST = S // 128     # 16 s-tiles
JT = S // 128     # 16 j-tiles
KT = D // 128     # 8 contraction tiles for the projections

# --- exp configuration -------------------------------------------------
A16 = 128.0 / np.log(2.0)          # bf16-bits per unit natural-log
AQ = A16 / 8.0                     # q prescale folding 1/sqrt(HD)
B16 = 16251.0                      # Schraudolph bias (127<<7 - tuning)
LNM_NEG = -20000.0                 # m=0 sentinel (wrap- and saturate-safe)
SCHRAUD = True                     # jt%4==3 tiles use DVE Schraudolph exp
GPSIMD_MULT = True                 # jt%4==1 tiles (after sc0) mult on GpSimd

_CACHED_NC = None
_LAST_RESULTS = None


def _build_bass():
    nc = bacc.Bacc("TRN2", target_bir_lowering=False, debug=False)

    xT_d = nc.dram_tensor("xT", [128, KT, S], BF16, kind="ExternalInput")
    wqk_d = nc.dram_tensor("wqk", [6, 128, KT * 128], BF16, kind="ExternalInput")
    wproj_d = nc.dram_tensor("wproj", [128, 2, D], BF16, kind="ExternalInput")
    mt_d = nc.dram_tensor("mt", [128, JT, S], BF16, kind="ExternalInput")
    bqk_d = nc.dram_tensor("bqk", [128, 6], F32, kind="ExternalInput")
    y_d = nc.dram_tensor("y", [KT, 128, S], BF16, kind="ExternalOutput")

    exp_scale = (1.0 / A16) if SCHRAUD else 0.125

    def is_schraud(jt):
        return SCHRAUD and jt % 4 == 3 and jt != 15

    def em_engine(scg, jt):
        if GPSIMD_MULT and scg > 0 and jt % 4 == 1:
            return nc.gpsimd
        return nc.vector

    with tile.TileContext(nc) as tc, ExitStack() as ctx:
        cp = ctx.enter_context(tc.tile_pool(name="consts", bufs=1))

        wqk_sb = cp.tile([128, KT, 3 * DH], BF16)
        wproj_sb = cp.tile([128, 2, D], BF16)
        bqk_sb = cp.tile([128, 6], F32)
        ones_f32 = cp.tile([128, 128], F32)
        qkT_sb = cp.tile([128, 4, S], BF16)  # nt 0,1 = qT; 2,3 = kT
        # [p, st, pair, hip, 128]: hip0 = [v|ones], hip1 = [ones|v]
        vext_sb = cp.tile([128, ST, 2, 2, 128], BF16)
        outT_sb = cp.tile([128, 2, S], BF16)     # proj lhsT layout

        xtp = tc.alloc_tile_pool(name="xtp", bufs=1)
        xT_sb = xtp.tile([128, KT, S], BF16)

        # ---- DMA wavefront: s-major so the pipeline starts early -------
        # scalar (HWDGE) queue: small consts + sc0 m-table
        nc.scalar.dma_start(bqk_sb[:], bqk_d[:])

        def load_wqk_block(eng, nt):
            eng.dma_start(
                wqk_sb[:, :, ts(nt, 128)],
                wqk_d[nt].rearrange("p (k c) -> p k c", c=128))

        load_wqk_block(nc.scalar, 2)      # k pair-0 weights first
        load_wqk_block(nc.scalar, 0)      # q pair-0

        # sync queue: xT in s-chunks, all kt of sq0 first
        for sq in range(4):
            for kt in range(KT):
                nc.sync.dma_start(xT_sb[:, kt, ts(sq, 512)],
                                  xT_d[:, kt, ts(sq, 512)])

        load_wqk_block(nc.scalar, 4)
        load_wqk_block(nc.scalar, 5)

        # sync queue, after all xT issues: pair-1 weights + wproj -- the
        # issue serialization keeps them off the xT wavefront's bandwidth
        load_wqk_block(nc.sync, 3)
        load_wqk_block(nc.sync, 1)
        nc.sync.dma_start(wproj_sb[:], wproj_d[:])

        # ---- constants + engine warmups during the DMA window ----------
        nc.vector.memset(ones_f32[:], 1.0)
        ones_bf = cp.tile([128, 128], BF16)
        nc.vector.memset(ones_bf[:], 1.0)
        nc.vector.memset(vext_sb[:, :, :, 0, HD:128], 1.0)
        nc.vector.memset(vext_sb[:, :, :, 1, 0:HD], 1.0)
        tblwarm = cp.tile([1, 8], F32)
        nc.scalar.activation(tblwarm[:], ones_f32[0:1, 0:8], Act.Exp)
        if GPSIMD_MULT:
            gwarm = cp.tile([1, 8], BF16)
            nc.gpsimd.tensor_tensor(gwarm[:], ones_bf[0:1, 0:8],
                                    ones_bf[0:1, 0:8], Alu.mult)

        pwarm = tc.alloc_tile_pool(name="pwarm", bufs=1, space="PSUM")
        warm = pwarm.tile([128, 128], F32, tag="warm", bufs=1)
        for _ in range(24):
            nc.tensor.matmul(warm[:], ones_bf[:], ones_bf[:],
                             start=True, stop=True, skip_group_check=True)
        pwarm.release()

        # ---- prologue: kT/qT for sq0 (kt-waves chasing the xT DMA) -----
        pbp = tc.alloc_tile_pool(name="pbp", bufs=1, space="PSUM")
        pk0 = pbp.tile([128, 512], F32, tag="pk0")
        pq0 = pbp.tile([128, 512], F32, tag="pq0")
        for kt in range(KT):
            nc.tensor.matmul(pk0[:], wqk_sb[:, kt, ts(2, 128)],
                             xT_sb[:, kt, 0:512],
                             start=(kt == 0), stop=(kt == KT - 1))
            nc.tensor.matmul(pq0[:], wqk_sb[:, kt, ts(0, 128)],
                             xT_sb[:, kt, 0:512],
                             start=(kt == 0), stop=(kt == KT - 1))
        nc.vector.tensor_scalar(qkT_sb[:, 2, 0:512], pk0[:],
                                bqk_sb[:, 2:3], None, Alu.add)
        if SCHRAUD:
            nc.vector.tensor_scalar(qkT_sb[:, 0, 0:512], pq0[:],
                                    bqk_sb[:, 0:1], AQ, Alu.add, Alu.mult)
        else:
            nc.vector.tensor_scalar(qkT_sb[:, 0, 0:512], pq0[:],
                                    bqk_sb[:, 0:1], None, Alu.add)
        pbp.release()

        dwork = tc.alloc_tile_pool(name="dwork", bufs=3)
        ystage = tc.alloc_tile_pool(name="ystage", bufs=3)

        # m-table: only one sc-chunk is live at a time; stream it (the
        # full table would cost 64KB/partition of SBUF)
        mtcs = {}

        def mtc_load(scg, eng, chunks=range(4)):
            sc = scg % 4
            t = mtcs.get(scg)
            if t is None:
                t = mtcs[scg] = dwork.tile([128, JT, 512], BF16, tag="mtc",
                                           bufs=2, name=f"mtc{scg}")
            for c4 in chunks:
                eng.dma_start(t[:, 4 * c4:4 * c4 + 4, :],
                              mt_d[:, 4 * c4:4 * c4 + 4, ts(sc, 512)])

        mtc_load(0, nc.scalar)
        pscp = tc.alloc_tile_pool(name="pscp", bufs=2, space="PSUM")
        po2p = tc.alloc_tile_pool(name="po2p", bufs=1, space="PSUM")
        aux = tc.alloc_tile_pool(name="aux0", bufs=1, space="PSUM")

        # ---- v extraction (c units): 8 kt-MMs + 2 casts ----------------
        def emit_c(st):
            pv = aux.tile([128, DH], F32, tag="pv", name=f"pv{st}")
            for kt in range(KT):
                nc.tensor.matmul(
                    pv[:], xT_sb[:, kt, ts(st, 128)],
                    wqk_sb[:, kt, 4 * 128:6 * 128],
                    start=(kt == 0), stop=(kt == KT - 1))
            pvv = pv[:].rearrange("p (pr hp d) -> p pr hp d", pr=2, hp=2)
            nc.vector.tensor_copy(vext_sb[:, st, :, 0, 0:HD], pvv[:, :, 0, :])
            nc.vector.tensor_copy(vext_sb[:, st, :, 1, HD:128], pvv[:, :, 1, :])

        emit_c(0)
        emit_c(1)

        # ---- qkv-projection waves (b units), 2 kt-MMs per step ---------
        bstate = {}

        def emit_b_piece(nt, sq, phase):
            if phase == 0:
                bstate[(nt, sq)] = aux.tile([128, 512], F32, tag="pq2",
                                            name=f"pq2_{nt}_{sq}")
            pq2 = bstate[(nt, sq)]
            for kt in range(2 * phase, 2 * phase + 2):
                nc.tensor.matmul(
                    pq2[:], wqk_sb[:, kt, ts(nt, 128)],
                    xT_sb[:, kt, ts(sq, 512)],
                    start=(kt == 0), stop=(kt == KT - 1))
            if phase == 3:
                if nt in (0, 1) and SCHRAUD:
                    nc.vector.tensor_scalar(
                        qkT_sb[:, nt, ts(sq, 512)], pq2[:],
                        bqk_sb[:, nt:nt + 1], AQ, Alu.add, Alu.mult)
                else:
                    nc.vector.tensor_scalar(
                        qkT_sb[:, nt, ts(sq, 512)], pq2[:],
                        bqk_sb[:, nt:nt + 1], None, Alu.add)

        # unit -> first global step; 4 consecutive steps each
        BSCHED = {
            (2, 1): 0, (2, 2): 4, (2, 3): 8,
            (0, 1): 12, (0, 2): 16, (0, 3): 20,
            (3, 0): 24, (3, 1): 28, (3, 2): 32, (3, 3): 36,
            (1, 0): 40, (1, 1): 44, (1, 2): 48, (1, 3): 52,
        }
        bsteps = {}
        for (nt, sq), st0 in BSCHED.items():
            for ph in range(4):
                bsteps.setdefault(st0 + ph, []).append((nt, sq, ph))

        # ---- output projection pieces ----------------------------------
        pstate = {}

        def emit_proj_mm(sc, g, phase, pool):
            nt = 2 * g + phase
            slot = pstate[(g, phase)] = pool.tile(
                [128, 512], F32, tag=f"py{phase}", name=f"py_{sc}_{nt}")
            for it in range(2):
                nc.tensor.matmul(
                    slot[:], wproj_sb[:, it, ts(nt, 128)],
                    outT_sb[:, it, ts(sc, 512)],
                    start=(it == 0), stop=(it == 1))

        def emit_proj_evac(sc, g):
            py0 = pstate.pop((g, 0))
            py1 = pstate.pop((g, 1))
            ysb = ystage.tile([128, 1024], BF16, tag="ysb", name=f"ysb{sc}_{g}")
            nc.scalar.copy(ysb[:, 0:512], py0[:])
            nc.vector.tensor_copy(ysb[:, 512:1024], py1[:])
            nc.sync.dma_start(y_d[2 * g, :, ts(sc, 512)], ysb[:, 0:512])
            nc.sync.dma_start(y_d[2 * g + 1, :, ts(sc, 512)], ysb[:, 512:1024])

        # ---- per-step primitives ---------------------------------------
        def emit_sc_mm(pair, sc, jt, warmfill=False):
            psc = pscp.tile([128, 1024], F32, tag="psc", name=f"psc{pair}{sc}{jt}")
            qT2 = qkT_sb[:, pair, :]
            kT2 = qkT_sb[:, 2 + pair, :]
            if warmfill:
                nc.tensor.matmul(psc[:, 0:512], ones_bf[:],
                                 qkT_sb[:, 0, 0:512], start=True, stop=True,
                                 skip_group_check=True)
            nc.tensor.matmul(psc[:, 0:512], kT2[0:64, ts(jt, 128)],
                             qT2[0:64, ts(sc, 512)], start=True, stop=True)
            nc.tensor.matmul(psc[:, 512:1024], kT2[64:128, ts(jt, 128)],
                             qT2[64:128, ts(sc, 512)], start=True, stop=True)
            return psc

        def emit_em(scg, sc, jt, psc):
            em = dwork.tile([128, 1024], BF16, tag="em", bufs=8,
                            name=f"em{scg}_{jt}")
            mtc = mtcs[scg][:, jt, :]
            if is_schraud(jt):
                lnm = mtc.bitcast(I16)
                mrep = lnm.unsqueeze(1).broadcast_to((128, 2, 512))
                nc.vector.tensor_tensor(
                    em[:].bitcast(U16).rearrange("p (t s) -> p t s", t=2),
                    psc[:].rearrange("p (t s) -> p t s", t=2),
                    mrep, Alu.add)
            else:
                eb = dwork.tile([128, 1024], BF16, tag="eb", bufs=4,
                                name=f"eb{scg}_{jt}")
                nc.scalar.activation(eb[:], psc[:], Act.Exp, scale=exp_scale)
                eng = em_engine(scg, jt)
                eng.tensor_tensor(em[:, 0:512], eb[:, 0:512], mtc, Alu.mult)
                eng.tensor_tensor(em[:, 512:1024], eb[:, 512:1024], mtc, Alu.mult)
            return em

        po2state = {}

        def emit_av(pair, scg, jt, em):
            po2a, po2b = po2state[scg]
            nc.tensor.matmul(po2a[:], vext_sb[:, jt, pair, 0, :],
                             em[:, 0:512],
                             start=(jt == 0), stop=(jt == JT - 1))
            nc.tensor.matmul(po2b[:], vext_sb[:, jt, pair, 1, :],
                             em[:, 512:1024],
                             start=(jt == 0), stop=(jt == JT - 1))

        def emit_norm(pair, sc, scg, piece):
            """normalize po2(scg) into outT; piece 0..3 across steps."""
            po2a, po2b = po2state[scg]
            s0 = sc * 512
            it = pair
            st = po2state.setdefault(("n", scg), {})
            if piece == 0:
                st["o2sa"] = dwork.tile([128, 512], F32, tag="o2sa", bufs=2, name=f"o2sa{scg}")
                st["o2sb"] = dwork.tile([128, 512], F32, tag="o2sb", bufs=2, name=f"o2sb{scg}")
                nc.scalar.copy(st["o2sa"][:], po2a[:])
                nc.scalar.copy(st["o2sb"][:], po2b[:])
            elif piece == 1:
                st["ra"] = dwork.tile([128, 512], F32, tag="rza", bufs=2, name=f"ra{scg}")
                st["rb"] = dwork.tile([128, 512], F32, tag="rzb", bufs=2, name=f"rb{scg}")
                nc.vector.reciprocal_approx_fast(out=st["ra"][:],
                                                 in_=st["o2sa"][:])
                nc.sync.dma_start(st["ra"][0:64, :], st["ra"][64:128, :])
                nc.vector.reciprocal_approx_fast(out=st["rb"][:],
                                                 in_=st["o2sb"][:])
                nc.sync.dma_start(st["rb"][64:128, :], st["rb"][0:64, :])
            elif piece == 2:
                nc.vector.tensor_tensor(
                    outT_sb[0:64, it, s0:s0 + 512],
                    st["o2sa"][0:64, :], st["ra"][0:64, :], Alu.mult)
                nc.vector.tensor_tensor(
                    outT_sb[64:128, it, s0:s0 + 512],
                    st["o2sb"][64:128, :], st["rb"][64:128, :], Alu.mult)
                del po2state[("n", scg)]

        # ---- the flat pipeline -----------------------------------------
        steps = [(pair, sc, jt)
                 for pair in range(2) for sc in range(4) for jt in range(JT)]
        pend = {}      # global step -> (pair, scg, jt, em)
        aux_released = False
        projp = None

        for g, (pair, sc, jt) in enumerate(steps):
            scg = pair * 4 + sc
            lstep = jt                     # local step within this sc

            # release phase-A aux slots once the last b unit is done;
            # the proj pool takes over the banks
            if g == 57 and not aux_released:
                aux.release()
                projp = tc.alloc_tile_pool(name="auxP", bufs=1, space="PSUM")
                aux_released = True

            # prefetch the next sc's m-table chunk (sync queue)
            if scg < 7 and lstep in (4, 7, 10, 13):
                mtc_load(scg + 1, nc.sync, chunks=((lstep - 4) // 3,))

            # normalize pieces for the previous sc
            if scg > 0 and 5 <= lstep <= 7:
                emit_norm(pair if sc > 0 else 1 - pair,
                          sc - 1 if sc > 0 else 3, scg - 1, lstep - 5)

            # output projection for sc-1 during pair-1 (local steps 8..15)
            if pair == 1 and sc > 0 and 8 <= lstep <= 15:
                pg, pph = divmod(lstep - 8, 2)
                if pph == 0:
                    emit_proj_mm(sc - 1, pg, 0, projp)
                else:
                    emit_proj_mm(sc - 1, pg, 1, projp)
                    emit_proj_evac(sc - 1, pg)

            # fresh po2 accumulators at the start of each sc
            if jt == 0:
                po2a = po2p.tile([128, 512], F32, tag="po2a", name=f"po2a{scg}")
                po2b = po2p.tile([128, 512], F32, tag="po2b", name=f"po2b{scg}")
                po2state[scg] = (po2a, po2b)

            # AV with lag 5 (crosses sc boundaries -> PE never drains;
            # the slack keeps the exp/em chain latency off the PE's back)
            if g - 5 in pend:
                p_pair, p_scg, p_jt, p_em = pend.pop(g - 5)
                emit_av(p_pair, p_scg, p_jt, p_em)

            psc = emit_sc_mm(pair, sc, jt, warmfill=(scg > 0))
            em = emit_em(scg, sc, jt, psc)
            pend[g] = (pair, scg, jt, em)

            # DMA-gated filler last, so it never head-of-line-blocks the
            # PE queue ahead of the independent SC/AV stream
            if scg == 0 and 1 <= lstep <= 14:
                emit_c(lstep + 1)
            for (nt, sq, ph) in bsteps.get(g, ()):
                emit_b_piece(nt, sq, ph)

        # ---- tail: last two AVs, last normalize, last projection -------
        # stack order: projp is on top -> it donates its banks to the tail
        projp.release()
        tailp = tc.alloc_tile_pool(name="tailp", bufs=1, space="PSUM")

        for gg in range(123, 128):
            p_pair, p_scg, p_jt, p_em = pend.pop(gg)
            emit_av(p_pair, p_scg, p_jt, p_em)

        s3 = 3 * 512
        tail0 = {}
        for nt in range(2):
            slot = tail0[nt] = tailp.tile([128, 512], F32, tag=f"pyt{nt}",
                                          name=f"pyt0_{nt}")
            nc.tensor.matmul(slot[:], wproj_sb[:, 0, ts(nt, 128)],
                             outT_sb[:, 0, s3:s3 + 512], start=True, stop=False)

        # normalize sc3 of pair-1 (staged, same as the in-loop pattern)
        po2a, po2b = po2state[7]
        o2ta = dwork.tile([128, 512], F32, tag="o2sa", bufs=2, name="o2ta")
        o2tb = dwork.tile([128, 512], F32, tag="o2sb", bufs=2, name="o2tb")
        nc.scalar.copy(o2ta[:], po2a[:])
        nc.vector.tensor_copy(o2tb[:], po2b[:])
        ra = dwork.tile([128, 512], F32, tag="rza", bufs=2, name="ratail")
        rb = dwork.tile([128, 512], F32, tag="rzb", bufs=2, name="rbtail")
        nc.vector.reciprocal_approx_fast(out=ra[:], in_=o2ta[:])
        nc.sync.dma_start(ra[0:64, :], ra[64:128, :])
        nc.vector.reciprocal_approx_fast(out=rb[:], in_=o2tb[:])
        nc.sync.dma_start(rb[64:128, :], rb[0:64, :])
        nc.vector.tensor_tensor(outT_sb[0:64, 1, s3:s3 + 512],
                                o2ta[0:64, :], ra[0:64, :], Alu.mult)
        nc.vector.tensor_tensor(outT_sb[64:128, 1, s3:s3 + 512],
                                o2tb[64:128, :], rb[64:128, :], Alu.mult)

        # finish the projection: it=1 accumulation + evac, two slots
        def tail_finish(nt, slot):
            nc.tensor.matmul(slot[:], wproj_sb[:, 1, ts(nt, 128)],
                             outT_sb[:, 1, s3:s3 + 512], start=False, stop=True)
            ysb = ystage.tile([128, 512], BF16, tag="ysb2", bufs=4,
                              name=f"ysbt{nt}")
            if nt % 2 == 0:
                nc.scalar.copy(ysb[:], slot[:])
            else:
                nc.vector.tensor_copy(ysb[:], slot[:])
            nc.sync.dma_start(y_d[nt, :, s3:s3 + 512], ysb[:])

        for nt in range(2):
            tail_finish(nt, tail0.pop(nt))
        for nt in range(2, 8):
            slot = tailp.tile([128, 512], F32, tag=f"pyt{nt % 2}",
                              name=f"pyt_{nt}")
            nc.tensor.matmul(slot[:], wproj_sb[:, 0, ts(nt, 128)],
                             outT_sb[:, 0, s3:s3 + 512], start=True, stop=False)
            tail_finish(nt, slot)

        tailp.release()
        po2p.release()
        pscp.release()
        ystage.release()
        dwork.release()
        xtp.release()

    nc.compile()
    return nc


def _get_nc():
    global _CACHED_NC
    if _CACHED_NC is None:
        _CACHED_NC = _build_bass()
    return _CACHED_NC


def _routes_payload(routes):
    """m-multiplicity table, transposed; Schraudolph rows carry the int16
    lnm-bias payload instead of bf16 m."""
    r = np.clip(np.asarray(routes).astype(np.int64), 0, S - 1)
    m = np.zeros((S, S), dtype=np.float64)
    np.add.at(m, (np.arange(S)[:, None].repeat(K, 1).ravel(), r.ravel()), 1.0)
    mT = m.T  # [j, s]
    payload = np.ascontiguousarray(mT.astype(bf16).reshape(JT, 128, S)
                                   .transpose(1, 0, 2))
    if SCHRAUD:
        lnm = np.where(mT > 0, A16 * np.log(np.maximum(mT, 1e-9)) + B16,
                       LNM_NEG)
        lnm_i16 = np.round(lnm).astype(np.int16)
        lnm_i16 = np.ascontiguousarray(lnm_i16.reshape(JT, 128, S)
                                       .transpose(1, 0, 2))
        for jt in range(JT):
            if jt % 4 == 3:
                payload[:, jt, :] = lnm_i16[:, jt, :].view(bf16)
    return payload


def _prep_core_inputs(x, W_qkv, b_qkv, W_proj, payload):
    maps = []
    for core in range(NCORES):
        b, hg = core // HG, core % HG
        c0 = hg * DH
        xT = np.ascontiguousarray(x[b].T).astype(bf16)            # (1024, 2048)
        wqk = np.concatenate(
            [W_qkv[:, c0:c0 + DH], W_qkv[:, D + c0:D + c0 + DH],
             W_qkv[:, 2 * D + c0:2 * D + c0 + DH]], axis=1)        # (1024, 768)
        bqk = np.concatenate([b_qkv[c0:c0 + DH], b_qkv[D + c0:D + c0 + DH],
                              b_qkv[2 * D + c0:2 * D + c0 + DH]])
        wproj = W_proj[c0:c0 + DH, :]                              # (256, 1024)
        maps.append({
            "xT": np.ascontiguousarray(xT.reshape(KT, 128, S).transpose(1, 0, 2)),
            "wqk": np.ascontiguousarray(
                wqk.astype(bf16).reshape(KT, 128, 6, 128)
                .transpose(2, 1, 0, 3).reshape(6, 128, KT * 128)),
            "wproj": np.ascontiguousarray(
                wproj.astype(bf16).reshape(2, 128, D).transpose(1, 0, 2)),
            "mt": payload,
            "bqk": np.ascontiguousarray(
                bqk.astype(np.float32).reshape(6, 128).T),
        })
    return maps


def kernel(x, W_qkv, b_qkv, W_proj, b_proj, routes):
    x = np.asarray(x, dtype=np.float32)
    W_qkv = np.asarray(W_qkv, dtype=np.float32)
    b_qkv = np.asarray(b_qkv, dtype=np.float32)
    W_proj = np.asarray(W_proj, dtype=np.float32)
    b_proj = np.asarray(b_proj, dtype=np.float32)

    payload = _routes_payload(routes)
    nc = _get_nc()
    in_maps = _prep_core_inputs(x, W_qkv, b_qkv, W_proj, payload)
    res = run_bass_kernel_spmd(nc, in_maps, core_ids=list(range(NCORES)))
    global _LAST_RESULTS
    _LAST_RESULTS = res

    y = np.zeros((B, S, D), dtype=np.float32)
    for core in range(NCORES):
        b = core // HG
        yT = res.results[core]["y"]          # (KT, 128, S) bf16, transposed
        y[b] += yT.astype(np.float32).reshape(D, S).T
    # device drops the v-bias; softmax weights sum to 1 so it adds exactly
    # b_v @ W_proj to every position
    y += (b_proj + b_qkv[2 * D:3 * D] @ W_proj)[None, None, :]
    return y
